# revision 1
# baseline (speedup 1.0000x reference)
"""GQA (grouped-query attention) Trainium2 kernel, SPMD across 8 NeuronCores.

Sharding: data-parallel over batch (B=2) x tensor-parallel over KV-head
groups (4 groups of 2 kv heads / 4 q heads). Core c handles batch c//4,
kv-group c%4. Each core computes its heads' attention plus a partial output
projection over its 512 context dims; the host sums the 4 partials per batch.

Device layout notes:
 - Q/K are produced transposed, (head_dim, seq), with head_dim de-interleaved
   (even dims in partitions 0..63, odd in 64..127) so RoPE acts on contiguous
   partition halves. Scores are computed transposed, (key_t, query_s), so the
   softmax denominator is a cross-partition sum done with an all-ones 128x128
   matmul on the TensorEngine (which also broadcasts it to all partitions).
 - V is produced as (seq, head_dim) natural order; context comes out
   (head_dim, seq), which directly feeds the output projection as lhsT.
 - exp() has no max-subtraction: scores/sqrt(128) have unit-ish scale after
   per-head RMS norm, so exp is safe in fp32, and softmax is shift-invariant.
"""

import os
import sys

import numpy as np
import ml_dtypes

for _p in ("/opt/trn_rl_repo",):
    if _p not in sys.path and os.path.isdir(_p):
        sys.path.insert(0, _p)

B, S, H, NH, G = 2, 2048, 2048, 16, 2
HD = H // NH          # 128 head dim
NKV = NH // G         # 8 kv heads
EPS = 1e-6
NCORES = 8
GROUPS = NCORES // B  # 4 kv-groups
NQH = NH // GROUPS    # 4 q heads per core
NKVH = NKV // GROUPS  # 2 kv heads per core
P = 128
SC = 512              # seq chunk
NSC = S // SC         # 4 chunks
KT = H // P           # 16 hidden k-tiles
TT = S // P           # 16 token tiles
HALF = HD // 2

BF16 = ml_dtypes.bfloat16
_NC_CACHE = {}


def _build_nc():
    import concourse.bass as bass  # noqa: F401
    import concourse.mybir as mybir
    import concourse.tile as tile
    from concourse import bacc

    fp32 = mybir.dt.float32
    bf16 = mybir.dt.bfloat16
    Alu = mybir.AluOpType
    Act = mybir.ActivationFunctionType

    nc = bacc.Bacc("TRN2", debug=False, enable_asserts=False, num_devices=NCORES)

    xT = nc.dram_tensor("xT", (H, S), bf16, kind="ExternalInput").ap()
    wq_d = nc.dram_tensor("wq", (H, NQH * HD), bf16, kind="ExternalInput").ap()
    wk_d = nc.dram_tensor("wk", (H, NKVH * HD), bf16, kind="ExternalInput").ap()
    wv_d = nc.dram_tensor("wv", (H, NKVH * HD), bf16, kind="ExternalInput").ap()
    wo_d = nc.dram_tensor("wo", (NQH * HD, H), bf16, kind="ExternalInput").ap()
    # doubled tables: cos2 = [cos; cos], sin2n = [-sin; +sin] so RoPE is
    # partition-aligned (DVE lanes cannot cross partitions)
    cos_d = nc.dram_tensor("cos2", (P, S), fp32, kind="ExternalInput").ap()
    sin_d = nc.dram_tensor("sin2n", (P, S), fp32, kind="ExternalInput").ap()
    mask_d = nc.dram_tensor("masks", (SC // P, P, SC), bf16, kind="ExternalInput").ap()
    rmsw_d = nc.dram_tensor("rmsw", (P, 1), fp32, kind="ExternalInput").ap()
    out_d = nc.dram_tensor("out", (S, H), fp32, kind="ExternalOutput").ap()

    inv_sqrt_hd = float(1.0 / np.sqrt(HD))

    with tile.TileContext(nc) as tc:
        with (
            tc.tile_pool(name="consts", bufs=1) as consts,
            tc.tile_pool(name="kv", bufs=1) as kv,
            tc.tile_pool(name="xp", bufs=2) as xp,
            tc.tile_pool(name="sq", bufs=2) as sqp,
            tc.tile_pool(name="rst", bufs=2) as rstp,
            tc.tile_pool(name="nrm", bufs=2) as nrmp,
            tc.tile_pool(name="rtmp", bufs=2) as rtmpp,
            tc.tile_pool(name="qr", bufs=2) as qrp,
            tc.tile_pool(name="ep", bufs=3) as ep,
            tc.tile_pool(name="rd", bufs=2) as rdp,
            tc.tile_pool(name="ctxn", bufs=5) as ctxp,
            tc.tile_pool(name="ob", bufs=3) as obp,
            tc.tile_pool(name="ps_proj", bufs=2, space="PSUM") as ps_proj,
            tc.tile_pool(name="ps_misc", bufs=2, space="PSUM") as ps_misc,
            tc.tile_pool(name="ps_sc", bufs=2, space="PSUM") as ps_sc,
            tc.tile_pool(name="ps_acc", bufs=1, space="PSUM") as ps_acc,
        ):
            # ---- resident constants ----
            wq_sb = consts.tile([P, KT, NQH * HD], bf16, name="wq_sb")
            wk_sb = consts.tile([P, KT, NKVH * HD], bf16, name="wk_sb")
            wv_sb = consts.tile([P, KT, NKVH * HD], bf16, name="wv_sb")
            wo_sb = consts.tile([P, NQH, H], bf16, name="wo_sb")
            cos_sb = consts.tile([P, S], fp32, name="cos_sb")
            sin_sb = consts.tile([P, S], fp32, name="sin_sb")
            mask_sb = consts.tile([P, SC // P, SC], bf16, name="mask_sb")
            ones_sb = consts.tile([P, P], bf16, name="ones_sb")
            rmsw_sb = consts.tile([P, 1], fp32, name="rmsw_sb")
            eps_sb = consts.tile([P, 1], fp32, name="eps_sb")

            wq_r = wq_d.rearrange("(kt p) c -> p kt c", p=P)
            wk_r = wk_d.rearrange("(kt p) c -> p kt c", p=P)
            wv_r = wv_d.rearrange("(kt p) c -> p kt c", p=P)
            for k in range(KT):
                nc.sync.dma_start(wv_sb[:, k, :], wv_r[:, k, :])
                nc.sync.dma_start(wk_sb[:, k, :], wk_r[:, k, :])
                nc.sync.dma_start(wq_sb[:, k, :], wq_r[:, k, :])
            wo_r = wo_d.rearrange("(h p) n -> p h n", p=P)
            for h in range(NQH):
                nc.sync.dma_start(wo_sb[:, h, :], wo_r[:, h, :])
            nc.sync.dma_start(cos_sb[:], cos_d)
            nc.sync.dma_start(sin_sb[:], sin_d)
            nc.sync.dma_start(mask_sb[:], mask_d.rearrange("j p c -> p j c"))
            nc.sync.dma_start(rmsw_sb[:], rmsw_d)
            nc.vector.memset(ones_sb[:], 1.0)
            nc.vector.memset(eps_sb[:], EPS)

            # full-sequence K (roped, transposed) and V caches
            kT_sb = kv.tile([P, NKVH, S], bf16, name="kT_sb")
            v_sb = kv.tile([P, TT, NKVH * HD], bf16, name="v_sb")

            xT_r = xT.rearrange("(kt p) s -> p kt s", p=P)

            def rms_norm(src_ps, dst, sl):
                """dst[128, SC] (f32) = src_ps * rms_w / sqrt(mean_d(src^2)+eps)."""
                sq = sqp.tile([P, SC], bf16, tag="sq")
                nc.scalar.activation(sq[:], src_ps[:], Act.Square)
                ms_ps = ps_misc.tile([P, SC], fp32, tag="misc")
                nc.tensor.matmul(ms_ps[:], ones_sb[:], sq[:], start=True, stop=True)
                rst = rstp.tile([P, SC], fp32, tag="rst")
                nc.scalar.activation(
                    rst[:], ms_ps[:], Act.Sqrt, scale=1.0 / HD, bias=eps_sb[:]
                )
                nc.vector.reciprocal(rst[:], rst[:])
                nc.vector.scalar_tensor_tensor(
                    dst[:], src_ps[:], rmsw_sb[:], rst[:], Alu.mult, Alu.mult
                )

            def rope(nrm, dst, sl):
                """dst[128, SC] (bf16) = rotate(nrm), partition-aligned form:
                dst = nrm * cos2 + swap_halves(nrm) * sin2n."""
                xs = rtmpp.tile([P, SC], fp32, tag="rt")
                nc.sync.dma_start(xs[0:HALF, :], nrm[HALF:P, :])
                nc.sync.dma_start(xs[HALF:P, :], nrm[0:HALF, :])
                nc.vector.tensor_mul(xs[:], xs[:], sin_sb[:, sl])
                nc.vector.tensor_mul(dst[:], nrm[:], cos_sb[:, sl])
                nc.vector.tensor_add(dst[:], dst[:], xs[:])

            for ci in range(NSC):
                sl = slice(ci * SC, (ci + 1) * SC)
                x_sb = xp.tile([P, KT, SC], bf16, tag="x")
                for k in range(KT):
                    nc.sync.dma_start(x_sb[:, k, :], xT_r[:, k, sl])

                # ---- V projection: (t, d) layout, both kv heads at once ----
                for tt in range(SC // P):
                    ti = ci * (SC // P) + tt
                    v_ps = ps_misc.tile([P, NKVH * HD], fp32, tag="misc")
                    tsl = slice(tt * P, (tt + 1) * P)
                    for k in range(KT):
                        nc.tensor.matmul(
                            v_ps[:],
                            x_sb[:, k, tsl],
                            wv_sb[:, k, :],
                            start=(k == 0),
                            stop=(k == KT - 1),
                        )
                    nc.any.tensor_copy(out=v_sb[:, ti, :], in_=v_ps[:])

                # ---- K projection + RMS + RoPE into the kv cache ----
                for lk in range(NKVH):
                    k_ps = ps_proj.tile([P, SC], fp32, tag="qk")
                    for k in range(KT):
                        nc.tensor.matmul(
                            k_ps[:],
                            wk_sb[:, k, lk * HD:(lk + 1) * HD],
                            x_sb[:, k, :],
                            start=(k == 0),
                            stop=(k == KT - 1),
                        )
                    knrm = nrmp.tile([P, SC], fp32, tag="nrm")
                    rms_norm(k_ps, knrm, sl)
                    rope(knrm, kT_sb[:, lk, sl], sl)

                # ---- Q per head: projection + RMS + RoPE + attention ----
                for lq in range(NQH):
                    lk = lq // 2
                    q_ps = ps_proj.tile([P, SC], fp32, tag="qk")
                    for k in range(KT):
                        nc.tensor.matmul(
                            q_ps[:],
                            wq_sb[:, k, lq * HD:(lq + 1) * HD],
                            x_sb[:, k, :],
                            start=(k == 0),
                            stop=(k == KT - 1),
                        )
                    qnrm = nrmp.tile([P, SC], fp32, tag="nrm")
                    rms_norm(q_ps, qnrm, sl)
                    qr = qrp.tile([P, SC], bf16, tag="qr")
                    rope(qnrm, qr, sl)

                    nt = (ci + 1) * (SC // P)
                    ctx_ps = ps_acc.tile([P, SC], fp32, tag="ctx")
                    den_ps = ps_acc.tile([P, SC], fp32, tag="den")

                    # scores pipelined one t-tile ahead of exp/ctx/den
                    sc_tiles = {}

                    def scores(tj):
                        sc_ps = ps_sc.tile([P, SC], fp32, tag="sc")
                        nc.tensor.matmul(
                            sc_ps[:],
                            kT_sb[:, lk, tj * P:(tj + 1) * P],
                            qr[:],
                            start=True,
                            stop=True,
                        )
                        sc_tiles[tj] = sc_ps

                    scores(0)
                    for tj in range(nt):
                        if tj + 1 < nt:
                            scores(tj + 1)
                        sc_ps = sc_tiles.pop(tj)
                        e = ep.tile([P, SC], bf16, tag="e")
                        nc.scalar.activation(
                            e[:], sc_ps[:], Act.Exp, scale=inv_sqrt_hd
                        )
                        if tj >= ci * (SC // P):
                            jj = tj - ci * (SC // P)
                            nc.vector.tensor_mul(e[:], e[:], mask_sb[:, jj, :])
                        nc.tensor.matmul(
                            ctx_ps[:],
                            v_sb[:, tj, lk * HD:(lk + 1) * HD],
                            e[:],
                            start=(tj == 0),
                            stop=(tj == nt - 1),
                        )
                        nc.tensor.matmul(
                            den_ps[:],
                            ones_sb[:],
                            e[:],
                            start=(tj == 0),
                            stop=(tj == nt - 1),
                        )

                    rd = rdp.tile([P, SC], fp32, tag="rd")
                    nc.vector.reciprocal(rd[:], den_ps[:])
                    ctxn = ctxp.tile([P, SC], bf16, tag=f"ctx{lq}")
                    nc.vector.tensor_mul(ctxn[:], ctx_ps[:], rd[:])
                    if lq == 0:
                        ctxn_tiles = {}
                    ctxn_tiles[lq] = ctxn

                # ---- output projection (partial over this core's 512 dims) ----
                for si in range(SC // P):
                    ssl = slice(si * P, (si + 1) * P)
                    for nj in range(H // SC):
                        o_ps = ps_sc.tile([P, SC], fp32, tag="sc")
                        for lq in range(NQH):
                            nc.tensor.matmul(
                                o_ps[:],
                                ctxn_tiles[lq][:, ssl],
                                wo_sb[:, lq, nj * SC:(nj + 1) * SC],
                                start=(lq == 0),
                                stop=(lq == NQH - 1),
                            )
                        ob = obp.tile([P, SC], fp32, tag="ob")
                        nc.any.tensor_copy(out=ob[:], in_=o_ps[:])
                        nc.sync.dma_start(
                            out_d[ci * SC + si * P:ci * SC + (si + 1) * P,
                                  nj * SC:(nj + 1) * SC],
                            ob[:],
                        )

    nc.compile()
    return nc


def get_nc():
    if "nc" not in _NC_CACHE:
        _NC_CACHE["nc"] = _build_nc()
    return _NC_CACHE["nc"]


def _d_perm():
    return np.concatenate([np.arange(0, HD, 2), np.arange(1, HD, 2)])


def make_core_inputs(x, wq, wk, wv, wo, rms_w, token_positions):
    """Build the 8 per-core input dicts (host-side shard + layout prep)."""
    d_perm = _d_perm()
    half = HD // 2
    inv_freq = 1.0 / (10000.0 ** (np.arange(half, dtype=np.float32) * 2.0 / HD))
    ang = token_positions.astype(np.float32)[:, None] * inv_freq[None, :]
    cosT = np.cos(ang).T.astype(np.float32)   # (64, S)
    sinT = np.sin(ang).T.astype(np.float32)
    cos2 = np.ascontiguousarray(np.vstack([cosT, cosT]))    # (128, S)
    sin2n = np.ascontiguousarray(np.vstack([-sinT, sinT]))  # (128, S)

    tt_idx = np.arange(P)[:, None]
    ss_idx = np.arange(SC)[None, :]
    masks = np.stack(
        [(jj * P + tt_idx <= ss_idx) for jj in range(SC // P)]
    ).astype(BF16)

    rmsw = np.ascontiguousarray(
        rms_w[d_perm].reshape(P, 1).astype(np.float32)
    )

    xT = [np.ascontiguousarray(x[b].T).astype(BF16) for b in range(B)]

    in_maps = []
    for c in range(NCORES):
        b, j = c // GROUPS, c % GROUPS
        q_cols = np.concatenate(
            [d_perm * NH + (NQH * j + lq) for lq in range(NQH)]
        )
        k_cols = np.concatenate(
            [d_perm * NKV + (NKVH * j + lk) for lk in range(NKVH)]
        )
        v_cols = np.concatenate(
            [np.arange(HD) * NKV + (NKVH * j + lk) for lk in range(NKVH)]
        )
        o_rows = np.concatenate(
            [(NQH * j + lq) * HD + np.arange(HD) for lq in range(NQH)]
        )
        in_maps.append({
            "xT": xT[b],
            "wq": np.ascontiguousarray(wq[:, q_cols]).astype(BF16),
            "wk": np.ascontiguousarray(wk[:, k_cols]).astype(BF16),
            "wv": np.ascontiguousarray(wv[:, v_cols]).astype(BF16),
            "wo": np.ascontiguousarray(wo[o_rows, :]).astype(BF16),
            "cos2": cos2,
            "sin2n": sin2n,
            "masks": masks,
            "rmsw": rmsw,
        })
    return in_maps


def kernel(**inputs):
    from concourse.bass_utils import run_bass_kernel_spmd

    x = np.asarray(inputs["x"], dtype=np.float32)
    wq = np.asarray(inputs["wq"], dtype=np.float32)
    wk = np.asarray(inputs["wk"], dtype=np.float32)
    wv = np.asarray(inputs["wv"], dtype=np.float32)
    wo = np.asarray(inputs["wo"], dtype=np.float32)
    rms_w = np.asarray(inputs["rms_w"], dtype=np.float32)
    pos = np.asarray(inputs["token_positions"])

    in_maps = make_core_inputs(x, wq, wk, wv, wo, rms_w, pos)
    nc = get_nc()
    res = run_bass_kernel_spmd(nc, in_maps, core_ids=list(range(NCORES)))
    out = np.zeros((B, S, H), np.float32)
    for c in range(NCORES):
        out[c // GROUPS] += res.results[c]["out"]
    return out



# revision 3
# speedup vs baseline: 3.7798x; 3.7798x over previous
"""GQA (grouped-query attention) Trainium2 kernel, SPMD across 8 NeuronCores.

Sharding: 8-way tensor-parallel over kv heads (core c owns kv head c and its
two grouped query heads 2c/2c+1) with both batches processed on every core.
The wall-clock of a warm call is dominated by host<->device transfer over the
axon tunnel, so the layout minimizes bytes moved:
 - x is uploaded seq-sharded (each core gets 1/8 of the tokens, 2MB bf16) and
   assembled on device with an AllGather -- 16MB total instead of 8MB/core.
 - Q/K/V/O weights are uploaded column/row-sharded per head (no duplication).
 - RoPE cos/sin tables are uploaded 1/8-sharded and AllGathered on device.
 - The output projection partials (each core covers 256 of the 2048
   contraction dims) are summed on device with a ReduceScatter, so each core
   returns only its 512x2048 slice of the final output, in bf16.

Device layout notes:
 - Q/K are produced transposed, (head_dim, seq), with head_dim de-interleaved
   (even dims in partitions 0..63, odd in 64..127) so RoPE acts on contiguous
   partition halves. Scores are computed transposed, (key_t, query_s), so the
   softmax denominator is a cross-partition sum done with an all-ones 128x128
   matmul on the TensorEngine (which also broadcasts it to all partitions).
 - V is produced as (seq, head_dim) natural order; context comes out
   (head_dim, seq), which directly feeds the output projection as lhsT.
 - exp() has no max-subtraction: scores/sqrt(128) have unit-ish scale after
   per-head RMS norm, so exp is safe in fp32, and softmax is shift-invariant.
"""

import os
import sys

import numpy as np
import ml_dtypes

for _p in ("/opt/trn_rl_repo",):
    if _p not in sys.path and os.path.isdir(_p):
        sys.path.insert(0, _p)

B, S, H, NH, G = 2, 2048, 2048, 16, 2
HD = H // NH          # 128 head dim
NKV = NH // G         # 8 kv heads
EPS = 1e-6
NCORES = 8
NQH = 2               # q heads per core
P = 128
SC = 512              # seq chunk
NSC = S // SC         # 4 chunks
KT = H // P           # 16 hidden k-tiles
HALF = HD // 2

BF16 = ml_dtypes.bfloat16
_NC_CACHE = {}


def _build_nc():
    import concourse.bass as bass  # noqa: F401
    import concourse.mybir as mybir
    import concourse.tile as tile
    from concourse import bacc

    fp32 = mybir.dt.float32
    bf16 = mybir.dt.bfloat16
    Alu = mybir.AluOpType
    Act = mybir.ActivationFunctionType

    nc = bacc.Bacc("TRN2", debug=False, enable_asserts=False, num_devices=NCORES)

    xpart_d = nc.dram_tensor("xpart", (H, SC), bf16, kind="ExternalInput").ap()
    tblpart_d = nc.dram_tensor("tblpart", (2 * P // NCORES, S), fp32,
                               kind="ExternalInput").ap()
    wq_d = nc.dram_tensor("wq", (H, NQH * HD), bf16, kind="ExternalInput").ap()
    wk_d = nc.dram_tensor("wk", (H, HD), bf16, kind="ExternalInput").ap()
    wv_d = nc.dram_tensor("wv", (H, HD), bf16, kind="ExternalInput").ap()
    wo_d = nc.dram_tensor("wo", (NQH * HD, H), bf16, kind="ExternalInput").ap()
    mask_d = nc.dram_tensor("masks", (SC // P, P, SC), bf16, kind="ExternalInput").ap()
    rmsw_d = nc.dram_tensor("rmsw", (P, 1), fp32, kind="ExternalInput").ap()
    outb_d = nc.dram_tensor("outb", (SC, H), bf16, kind="ExternalOutput").ap()

    inv_sqrt_hd = float(1.0 / np.sqrt(HD))

    from contextlib import ExitStack

    with tile.TileContext(nc) as tc:
        with ExitStack() as stack:
            ent = stack.enter_context
            dram = ent(tc.tile_pool(name="dram", bufs=1, space="DRAM"))
            consts = ent(tc.tile_pool(name="consts", bufs=1))
            kv = ent(tc.tile_pool(name="kv", bufs=1))
            xp = ent(tc.tile_pool(name="xp", bufs=2))
            sqp = ent(tc.tile_pool(name="sq", bufs=2))
            rstp = ent(tc.tile_pool(name="rst", bufs=2))
            nrmp = ent(tc.tile_pool(name="nrm", bufs=2))
            rtmpp = ent(tc.tile_pool(name="rtmp", bufs=2))
            qrp = ent(tc.tile_pool(name="qr", bufs=2))
            ep = ent(tc.tile_pool(name="ep", bufs=3))
            rdp = ent(tc.tile_pool(name="rd", bufs=2))
            ctxp = ent(tc.tile_pool(name="ctxn", bufs=3))
            obp = ent(tc.tile_pool(name="ob", bufs=3))
            finp = ent(tc.tile_pool(name="fin", bufs=2))
            ps_proj = ent(tc.tile_pool(name="ps_proj", bufs=2, space="PSUM"))
            ps_misc = ent(tc.tile_pool(name="ps_misc", bufs=2, space="PSUM"))
            ps_sc = ent(tc.tile_pool(name="ps_sc", bufs=2, space="PSUM"))
            ps_acc = ent(tc.tile_pool(name="ps_acc", bufs=1, space="PSUM"))
            # ---- DRAM staging for collectives ----
            xg_in = dram.tile([H, SC], bf16, name="xg_in")
            xg = dram.tile([NCORES * H, SC], bf16, name="xg", addr_space="Shared")
            tbl_in = dram.tile([2 * P // NCORES, S], fp32, name="tbl_in")
            tbl = dram.tile([2 * P, S], fp32, name="tbl", addr_space="Shared")
            part = dram.tile([B * S, H], fp32, name="part")
            rs_o = dram.tile([SC, H], fp32, name="rs_o")

            grp = [list(range(NCORES))]
            nc.sync.dma_start(xg_in[:], xpart_d)
            nc.gpsimd.collective_compute(
                "AllGather", Alu.bypass, replica_groups=grp,
                ins=[xg_in.opt()], outs=[xg.opt()],
            )
            nc.sync.dma_start(tbl_in[:], tblpart_d)
            nc.gpsimd.collective_compute(
                "AllGather", Alu.bypass, replica_groups=grp,
                ins=[tbl_in.opt()], outs=[tbl.opt()],
            )

            # ---- resident constants ----
            wq_sb = consts.tile([P, KT, NQH * HD], bf16, name="wq_sb")
            wk_sb = consts.tile([P, KT, HD], bf16, name="wk_sb")
            wv_sb = consts.tile([P, KT, HD], bf16, name="wv_sb")
            wo_sb = consts.tile([P, NQH, H], bf16, name="wo_sb")
            cos_sb = consts.tile([P, S], fp32, name="cos_sb")
            sin_sb = consts.tile([P, S], fp32, name="sin_sb")
            mask_sb = consts.tile([P, SC // P, SC], bf16, name="mask_sb")
            ones_sb = consts.tile([P, P], bf16, name="ones_sb")
            rmsw_sb = consts.tile([P, 1], fp32, name="rmsw_sb")
            eps_sb = consts.tile([P, 1], fp32, name="eps_sb")

            wq_r = wq_d.rearrange("(kt p) c -> p kt c", p=P)
            wk_r = wk_d.rearrange("(kt p) c -> p kt c", p=P)
            wv_r = wv_d.rearrange("(kt p) c -> p kt c", p=P)
            for k in range(KT):
                nc.sync.dma_start(wv_sb[:, k, :], wv_r[:, k, :])
                nc.sync.dma_start(wk_sb[:, k, :], wk_r[:, k, :])
                nc.sync.dma_start(wq_sb[:, k, :], wq_r[:, k, :])
            wo_r = wo_d.rearrange("(h p) n -> p h n", p=P)
            for h in range(NQH):
                nc.sync.dma_start(wo_sb[:, h, :], wo_r[:, h, :])
            nc.sync.dma_start(cos_sb[:], tbl[0:P, :])
            nc.sync.dma_start(sin_sb[:], tbl[P:2 * P, :])
            nc.sync.dma_start(mask_sb[:], mask_d.rearrange("j p c -> p j c"))
            nc.sync.dma_start(rmsw_sb[:], rmsw_d)
            nc.vector.memset(ones_sb[:], 1.0)
            nc.vector.memset(eps_sb[:], EPS)

            def rms_norm(src_ps, dst):
                """dst[128, SC] (f32) = src_ps * rms_w / sqrt(mean_d(src^2)+eps)."""
                sq = sqp.tile([P, SC], bf16, tag="sq")
                nc.scalar.activation(sq[:], src_ps[:], Act.Square)
                ms_ps = ps_misc.tile([P, SC], fp32, tag="misc")
                nc.tensor.matmul(ms_ps[:], ones_sb[:], sq[:], start=True, stop=True)
                rst = rstp.tile([P, SC], fp32, tag="rst")
                nc.scalar.activation(
                    rst[:], ms_ps[:], Act.Sqrt, scale=1.0 / HD, bias=eps_sb[:]
                )
                nc.vector.reciprocal(rst[:], rst[:])
                nc.vector.scalar_tensor_tensor(
                    dst[:], src_ps[:], rmsw_sb[:], rst[:], Alu.mult, Alu.mult
                )

            def rope(nrm, dst, sl):
                """dst[128, SC] (bf16) = rotate(nrm), partition-aligned form:
                dst = nrm * cos2 + swap_halves(nrm) * sin2n."""
                xs = rtmpp.tile([P, SC], fp32, tag="rt")
                nc.sync.dma_start(xs[0:HALF, :], nrm[HALF:P, :])
                nc.sync.dma_start(xs[HALF:P, :], nrm[0:HALF, :])
                nc.vector.tensor_mul(xs[:], xs[:], sin_sb[:, sl])
                nc.vector.tensor_mul(dst[:], nrm[:], cos_sb[:, sl])
                nc.vector.tensor_add(dst[:], dst[:], xs[:])

            for b in range(B):
                # full-sequence K (roped, transposed) and V caches for batch b
                kT_sb = kv.tile([P, S], bf16, tag=f"kT{b}")
                v_sb = kv.tile([P, S // P, HD], bf16, tag=f"v{b}")

                for ci in range(NSC):
                    sl = slice(ci * SC, (ci + 1) * SC)
                    x_sb = xp.tile([P, KT, SC], bf16, tag="x")
                    xoff = (b * NSC + ci) * H
                    for k in range(KT):
                        nc.sync.dma_start(
                            x_sb[:, k, :], xg[xoff + k * P: xoff + (k + 1) * P, :]
                        )

                    # ---- V projection: (t, d) layout ----
                    for tt in range(SC // P):
                        ti = ci * (SC // P) + tt
                        v_ps = ps_misc.tile([P, HD], fp32, tag="misc")
                        tsl = slice(tt * P, (tt + 1) * P)
                        for k in range(KT):
                            nc.tensor.matmul(
                                v_ps[:],
                                x_sb[:, k, tsl],
                                wv_sb[:, k, :],
                                start=(k == 0),
                                stop=(k == KT - 1),
                            )
                        nc.any.tensor_copy(out=v_sb[:, ti, :], in_=v_ps[:])

                    # ---- K projection + RMS + RoPE into the kv cache ----
                    k_ps = ps_proj.tile([P, SC], fp32, tag="qk")
                    for k in range(KT):
                        nc.tensor.matmul(
                            k_ps[:],
                            wk_sb[:, k, :],
                            x_sb[:, k, :],
                            start=(k == 0),
                            stop=(k == KT - 1),
                        )
                    knrm = nrmp.tile([P, SC], fp32, tag="nrm")
                    rms_norm(k_ps, knrm)
                    rope(knrm, kT_sb[:, sl], sl)

                    # ---- Q per head: projection + RMS + RoPE + attention ----
                    ctxn_tiles = {}
                    for lq in range(NQH):
                        q_ps = ps_proj.tile([P, SC], fp32, tag="qk")
                        for k in range(KT):
                            nc.tensor.matmul(
                                q_ps[:],
                                wq_sb[:, k, lq * HD:(lq + 1) * HD],
                                x_sb[:, k, :],
                                start=(k == 0),
                                stop=(k == KT - 1),
                            )
                        qnrm = nrmp.tile([P, SC], fp32, tag="nrm")
                        rms_norm(q_ps, qnrm)
                        qr = qrp.tile([P, SC], bf16, tag="qr")
                        rope(qnrm, qr, sl)

                        nt = (ci + 1) * (SC // P)
                        ctx_ps = ps_acc.tile([P, SC], fp32, tag="ctx")
                        den_ps = ps_acc.tile([P, SC], fp32, tag="den")

                        # scores pipelined one t-tile ahead of exp/ctx/den
                        sc_tiles = {}

                        def scores(tj):
                            sc_ps = ps_sc.tile([P, SC], fp32, tag="sc")
                            nc.tensor.matmul(
                                sc_ps[:],
                                kT_sb[:, tj * P:(tj + 1) * P],
                                qr[:],
                                start=True,
                                stop=True,
                            )
                            sc_tiles[tj] = sc_ps

                        scores(0)
                        for tj in range(nt):
                            if tj + 1 < nt:
                                scores(tj + 1)
                            sc_ps = sc_tiles.pop(tj)
                            e = ep.tile([P, SC], bf16, tag="e")
                            nc.scalar.activation(
                                e[:], sc_ps[:], Act.Exp, scale=inv_sqrt_hd
                            )
                            if tj >= ci * (SC // P):
                                jj = tj - ci * (SC // P)
                                nc.vector.tensor_mul(e[:], e[:], mask_sb[:, jj, :])
                            nc.tensor.matmul(
                                ctx_ps[:],
                                v_sb[:, tj, :],
                                e[:],
                                start=(tj == 0),
                                stop=(tj == nt - 1),
                            )
                            nc.tensor.matmul(
                                den_ps[:],
                                ones_sb[:],
                                e[:],
                                start=(tj == 0),
                                stop=(tj == nt - 1),
                            )

                        rd = rdp.tile([P, SC], fp32, tag="rd")
                        nc.vector.reciprocal(rd[:], den_ps[:])
                        ctxn = ctxp.tile([P, SC], bf16, tag=f"ctx{lq}")
                        nc.vector.tensor_mul(ctxn[:], ctx_ps[:], rd[:])
                        ctxn_tiles[lq] = ctxn

                    # ---- partial output projection over this core's 256 dims ----
                    for si in range(SC // P):
                        ssl = slice(si * P, (si + 1) * P)
                        row0 = b * S + ci * SC + si * P
                        for nj in range(H // SC):
                            o_ps = ps_sc.tile([P, SC], fp32, tag="sc")
                            for lq in range(NQH):
                                nc.tensor.matmul(
                                    o_ps[:],
                                    ctxn_tiles[lq][:, ssl],
                                    wo_sb[:, lq, nj * SC:(nj + 1) * SC],
                                    start=(lq == 0),
                                    stop=(lq == NQH - 1),
                                )
                            ob = obp.tile([P, SC], fp32, tag="ob")
                            nc.any.tensor_copy(out=ob[:], in_=o_ps[:])
                            nc.sync.dma_start(
                                part[row0:row0 + P, nj * SC:(nj + 1) * SC],
                                ob[:],
                            )

            # ---- sum partials across cores; each core keeps its 512 rows ----
            nc.gpsimd.collective_compute(
                "ReduceScatter", Alu.add, replica_groups=grp,
                ins=[part.opt()], outs=[rs_o.opt()],
            )
            for si in range(SC // P):
                t32 = finp.tile([P, H], fp32, tag="f32")
                nc.sync.dma_start(t32[:], rs_o[si * P:(si + 1) * P, :])
                t16 = finp.tile([P, H], bf16, tag="f16")
                nc.any.tensor_copy(out=t16[:], in_=t32[:])
                nc.sync.dma_start(outb_d[si * P:(si + 1) * P, :], t16[:])

    nc.compile()
    return nc


def get_nc():
    if "nc" not in _NC_CACHE:
        _NC_CACHE["nc"] = _build_nc()
    return _NC_CACHE["nc"]


def _d_perm():
    return np.concatenate([np.arange(0, HD, 2), np.arange(1, HD, 2)])


def make_core_inputs(x, wq, wk, wv, wo, rms_w, token_positions):
    """Build the 8 per-core input dicts (host-side shard + layout prep)."""
    d_perm = _d_perm()
    half = HD // 2
    inv_freq = 1.0 / (10000.0 ** (np.arange(half, dtype=np.float32) * 2.0 / HD))
    ang = token_positions.astype(np.float32)[:, None] * inv_freq[None, :]
    cosT = np.cos(ang).T.astype(np.float32)   # (64, S)
    sinT = np.sin(ang).T.astype(np.float32)
    # doubled tables: cos2 = [cos; cos], sin2n = [-sin; +sin] so RoPE is
    # partition-aligned (DVE lanes cannot cross partitions); stacked
    # (256, S) and uploaded 1/8 per core for the on-device AllGather
    tbl = np.vstack([cosT, cosT, -sinT, sinT]).astype(np.float32)

    tt_idx = np.arange(P)[:, None]
    ss_idx = np.arange(SC)[None, :]
    masks = np.stack(
        [(jj * P + tt_idx <= ss_idx) for jj in range(SC // P)]
    ).astype(BF16)

    rmsw = np.ascontiguousarray(
        rms_w[d_perm].reshape(P, 1).astype(np.float32)
    )

    trows = (2 * P) // NCORES
    in_maps = []
    for c in range(NCORES):
        b, qi = c // NSC, c % NSC
        q_cols = np.concatenate(
            [d_perm * NH + (NQH * c + lq) for lq in range(NQH)]
        )
        k_cols = d_perm * NKV + c
        v_cols = np.arange(HD) * NKV + c
        o_rows = np.concatenate(
            [(NQH * c + lq) * HD + np.arange(HD) for lq in range(NQH)]
        )
        in_maps.append({
            "xpart": x[b].T[:, qi * SC:(qi + 1) * SC].astype(BF16),
            "tblpart": tbl[c * trows:(c + 1) * trows],
            "wq": wq[:, q_cols].astype(BF16),
            "wk": wk[:, k_cols].astype(BF16),
            "wv": wv[:, v_cols].astype(BF16),
            "wo": wo[o_rows, :].astype(BF16),
            "masks": masks,
            "rmsw": rmsw,
        })
    return in_maps


def gather_output(results):
    out = np.empty((B, S, H), np.float32)
    for c in range(NCORES):
        b, qi = c // NSC, c % NSC
        out[b, qi * SC:(qi + 1) * SC] = results[c]["outb"].astype(np.float32)
    return out


def kernel(**inputs):
    from concourse.bass_utils import run_bass_kernel_spmd

    x = np.asarray(inputs["x"], dtype=np.float32)
    wq = np.asarray(inputs["wq"], dtype=np.float32)
    wk = np.asarray(inputs["wk"], dtype=np.float32)
    wv = np.asarray(inputs["wv"], dtype=np.float32)
    wo = np.asarray(inputs["wo"], dtype=np.float32)
    rms_w = np.asarray(inputs["rms_w"], dtype=np.float32)
    pos = np.asarray(inputs["token_positions"])

    in_maps = make_core_inputs(x, wq, wk, wv, wo, rms_w, pos)
    nc = get_nc()
    res = run_bass_kernel_spmd(nc, in_maps, core_ids=list(range(NCORES)))
    return gather_output(res.results)
